# revision 29
# baseline (speedup 1.0000x reference)
"""Trainium2 Bass kernel for nn_DecoderBlock (dense transformer decoder block).

Sharding: data-parallel over batch N=8 -> one batch element per NeuronCore.
Zero collectives; weights replicated to every core.

Per-core computation (K=1024 tokens, M=1024 emb, H=8 heads, DH=128, FF=4096):
  a1 = MHA(dec, dec);  x1 = LN(dec + a1)
  a2 = MHA(x1, enc);   x2 = LN(x1 + a2)
  ff = relu(x2 @ W1.T) @ W2.T;  out = LN(x2 + ff)

QKV/scores/Wo/W1 matmuls run in float32r (fp32 rounded to 11 mantissa bits;
full PE speed at free-dim >= 256); the softmax-exp weights, V, and the FFN
relu/W2 path run in bf16 (mixed 32/16-bit matmul operands are rejected by
the compiler, so each matmul is uniformly f32r or uniformly bf16).
LN/residual data stays fp32.

Layout strategy:
  - Activations kept in natural [k, m] layout for LN/residual (per-partition
    row stats) and in transposed [m, k] layout (f32r) as matmul operands.
  - Attention computes scores^T (keys on partitions). exp runs on kt-pairs
    ([P,1024] ACT ops halve the fixed overhead); bf16 expq double-buffers
    consecutive softmax units. The softmax denominator comes from a
    ones-lhsT matmul that simultaneously broadcasts it to all 128
    partitions; y^T per head directly feeds the Wo matmul as lhsT, so the
    attention matrix is never transposed.
  - Only two PE-transpose passes exist (x1 -> x1T, x2 -> x2T); their
    PSUM->SBUF casts run on the Scalar engine to keep DVE free for LN.

Scheduling:
  - Queue split: weight DMAs on the Sync HWDGE queue, activation loads +
    output stores on the Scalar HWDGE queue, x1/x2 scratch stores on the
    GpSimd SWDGE queue -- a store waiting on an LN result never
    head-of-line-blocks a weight prefetch dispatch.
  - Input loads are split per kt-slice so compute starts after ~1/8 of the
    input lands; enc's load is deferred into sa_att (post_head hook).
  - Deep weight-prefetch rings (qk 6, wo 8, w1 6) keep the PE fed; W1
    streams once, feeding both k-halves from each weight tile.
  - v-projections of the NEXT attention run as PE filler inside the
    current Wo+LN block, emitted before the final transpose (the PE queue
    is in-order, so filler must precede instructions that wait on DVE).
Host pre-packs/transposes all weights so every DMA row is 2-4KB contiguous.
"""
import sys

sys.path.insert(0, "/opt/trn_rl_repo")

import numpy as np
import ml_dtypes

# antenv.axon_hooks shim (needed only if BASS_TRACE is set; the agent image's
# read-only antenv package lacks this module).
try:
    from antenv import axon_hooks as _ah  # noqa: F401
except ImportError:
    import types as _types

    _h = _types.ModuleType("antenv.axon_hooks")
    _h._HOOK = None

    def _get_hook():
        if _h._HOOK is None:
            try:
                from trn_agent_boot.trn_boot import _ntff_profile_via_ctypes

                _h._HOOK = _ntff_profile_via_ctypes("/opt/axon/libaxon_pjrt.so")
            except Exception:
                _h._HOOK = None
        return _h._HOOK

    _h.get_axon_ntff_profile_hook = _get_hook
    _h.set_axon_ntff_profile_hook = lambda hook: setattr(_h, "_HOOK", hook)
    sys.modules["antenv.axon_hooks"] = _h

import concourse.bass as bass
import concourse.tile as tile
from concourse import bacc, mybir
from concourse.bass_utils import run_bass_kernel_spmd
from concourse.masks import make_identity

F32 = mybir.dt.float32
F32R = mybir.dt.float32r
BF16 = mybir.dt.bfloat16
AF = mybir.ActivationFunctionType
OP = mybir.AluOpType

P = 128          # partitions
K = 1024         # sequence length
M = 1024         # embedding dim
H = 8            # heads
DH = 128         # head dim
HD = H * DH      # 1024
FF = 4096
KT = K // P      # 8 seq tiles
MT = M // P      # 8 emb tiles
HT = HD // P     # 8 hd tiles
FT = FF // P     # 32 ff tiles
NQ = 2           # k_q halves (free dim 512 keeps fp32r at full PE speed)
QW = K // NQ     # 512
EPS = 1e-10
ISQ = 1.0 / float(np.sqrt(DH))

N_CORES = 8
USE_SCALAR_QUEUE = True


def round_fp32r(x: np.ndarray) -> np.ndarray:
    """Round fp32 to fp32r (11-bit mantissa, RNE), fp32 container."""
    b = np.ascontiguousarray(x, dtype=np.float32).view(np.uint32)
    lsb = (b >> 12) & 1
    out = (b + 0x7FF + lsb) & 0xFFFFF000
    return out.view(np.float32)


def _bcast_row_ap(t: bass.AP, width: int) -> bass.AP:
    """DRAM vector -> AP broadcasting one row across 128 partitions."""
    return bass.AP(tensor=t.tensor, offset=t.offset, ap=[[0, P], [1, width]])



def _ldq(nc):
    """Queue for activation loads/stores (scalar HWDGE when enabled)."""
    return nc.scalar if USE_SCALAR_QUEUE else nc.sync


def build_kernel(flags: dict):
    nc = bacc.Bacc("TRN2", target_bir_lowering=False, debug=False,
                   num_devices=N_CORES)
    dram = {}

    def din(name, shape, dt=F32R):
        dram[name] = nc.dram_tensor(name, shape, dt, kind="ExternalInput").ap()

    din("xt_dec", (P, MT * K), BF16)
    din("xt_enc", (P, MT * K), BF16)
    din("wq_sa", (H, P, MT * DH), BF16); din("wk_sa", (H, P, MT * DH), BF16)
    din("wq_ca", (H, P, MT * DH), BF16); din("wk_ca", (H, P, MT * DH), BF16)
    din("wv_sa", (MT, P, HD), BF16); din("wv_ca", (MT, P, HD), BF16)
    din("wo_sa", (HT, P, M), BF16); din("wo_ca", (HT, P, M), BF16)
    din("w1", (FT, P, MT * P), BF16); din("w2", (FT, P, M), BF16)
    din("dec_nat", (K, M), F32)
    for nm in ("bq_sa", "bk_sa", "bq_ca", "bk_ca"):
        if flags[nm]:
            din(nm, (DH, H), F32)
    for nm in ("bv_sa", "bv_ca", "bo_sa", "bo_ca", "bf2",
               "g1", "b1", "g2", "b2", "g3", "b3"):
        if flags[nm]:
            din(nm, (M,), F32)
    if flags["bf1"]:
        din("bf1", (P, FT), F32)
    out = nc.dram_tensor("out", (K, M), F32, kind="ExternalOutput").ap()

    with tile.TileContext(nc) as tc:
        _emit(nc, tc, dram, out, flags)
    nc.compile()
    return nc


def _emit(nc, tc, dram, out, flags):
    from contextlib import ExitStack

    with ExitStack() as ctx:
        # ---------- persistent pools ----------
        const = ctx.enter_context(tc.tile_pool(name="const", bufs=1))
        natp = ctx.enter_context(tc.tile_pool(name="natp", bufs=2))
        residp = ctx.enter_context(tc.tile_pool(name="residp", bufs=2))
        statp = ctx.enter_context(tc.tile_pool(name="statp", bufs=4))
        xpool = ctx.enter_context(tc.tile_pool(name="xpool", bufs=1))
        ps = ctx.enter_context(tc.tile_pool(name="ps", bufs=4, space="PSUM"))
        dscr = ctx.enter_context(tc.tile_pool(name="dscr", bufs=1,
                                              space="DRAM"))

        # queue split: weights go on the Sync HWDGE queue; activation
        # loads/stores on the Scalar HWDGE queue, so a store waiting on an
        # LN result never head-of-line-blocks a weight prefetch dispatch.
        ones_t = const.tile([P, P], F32R, name="ones_t")
        ones_f = const.tile([P, P], F32, name="ones_f")
        nc.vector.memset(ones_f, 1.0)
        nc.vector.tensor_copy(ones_t, ones_f)
        ones_b = const.tile([P, P], BF16, name="ones_b")
        nc.vector.memset(ones_b, 1.0)
        ident = const.tile([P, P], F32, name="ident")
        make_identity(nc, ident)
        eps_t = const.tile([P, 1], F32, name="eps_t")
        nc.vector.memset(eps_t, EPS)

        bias_tiles = {}
        for nm in ("bq_sa", "bk_sa", "bq_ca", "bk_ca"):
            if flags[nm]:
                t = const.tile([P, H], F32, name=nm + "_t")
                _ldq(nc).dma_start(out=t, in_=dram[nm])
                bias_tiles[nm] = t
        if flags["bf1"]:
            t = const.tile([P, FT], F32, name="bf1_t")
            _ldq(nc).dma_start(out=t, in_=dram["bf1"])
            bias_tiles["bf1"] = t
        for nm in ("bv_sa", "bv_ca", "bo_sa", "bo_ca", "bf2",
                   "g1", "b1", "g2", "b2", "g3", "b3"):
            if flags[nm]:
                t = const.tile([P, M], F32, name=nm + "_t")
                _ldq(nc).dma_start(out=t, in_=_bcast_row_ap(dram[nm], M))
                bias_tiles[nm] = t

        x1_store = dscr.tile([K, M], F32, name="x1_store")
        x2_store = dscr.tile([K, M], F32, name="x2_store")

        def new_xt(name):
            return xpool.tile([P, MT, K], BF16, name=name, tag="xt_slot")

        # input loads split per kt-slice so the first v-projection chunk can
        # start once ~1/8 of the input has landed
        xt = new_xt("decT")
        xt_src = dram["xt_dec"].rearrange("p (mt k) -> p mt k", mt=MT)
        for kt in range(KT):
            _ldq(nc).dma_start(out=xt[:, :, kt * P:(kt + 1) * P],
                                in_=xt_src[:, :, kt * P:(kt + 1) * P])

        # ---------------- building blocks ----------------
        def v_proj_chunks(wvp, src_xt, wv_name, bv_name, vcat, W=256):
            """Emission callbacks for the Wv projection: for each hd slice of
            width W, a weight-load callback then 8 per-kt psum-chain
            callbacks. The caller may interleave them with other PE work."""
            state = {}
            cbs = []
            NG = HD // W

            def make_load(g):
                def load():
                    tiles = []
                    for mt in range(MT):
                        w = wvp.tile([P, W], BF16, name=f"wv{g}_{mt}",
                                     tag="wvh", bufs=8)
                        nc.sync.dma_start(
                            out=w,
                            in_=dram[wv_name][mt, :, g * W:(g + 1) * W])
                        tiles.append(w)
                    state[g] = tiles
                return load

            def make_chunk(g, kt):
                def chunk():
                    wvts = state[g]
                    pv = ps.tile([P, W], F32, name=f"psv{g}_{kt}", tag="ps")
                    for mt in range(MT):
                        nc.tensor.matmul(
                            pv, src_xt[:, mt, kt * P:(kt + 1) * P],
                            wvts[mt], start=(mt == 0), stop=(mt == MT - 1))
                    dst = vcat[:, kt, g * W:(g + 1) * W]
                    if flags[bv_name]:
                        nc.vector.scalar_tensor_tensor(
                            out=dst, in0=pv, scalar=1.0,
                            in1=bias_tiles[bv_name][:, g * W:(g + 1) * W],
                            op0=OP.bypass, op1=OP.add)
                    else:
                        # ACT copy keeps the DVE queue free for LN chains
                        nc.scalar.activation(dst, pv, AF.Copy)
                return chunk

            for g in range(NG):
                cbs.append(make_load(g))
                for kt in range(KT):
                    cbs.append(make_chunk(g, kt))
            return cbs

        def qk_head_proj(wtq, src_xt, w_name, b_name, h, dst):
            """dst[p(d), k] (f32r) = head-h projection of src (+bias col)."""
            w = wtq.tile([P, MT, DH], BF16, name=f"{w_name}_{h}", tag="wt")
            nc.sync.dma_start(out=w, in_=dram[w_name][h].rearrange(
                "p (mt d) -> p mt d", mt=MT))
            for kh in range(2):
                pq = ps.tile([P, 512], F32, name=f"pq_{w_name}_{h}_{kh}",
                             tag="ps")
                for mt in range(MT):
                    nc.tensor.matmul(
                        pq, w[:, mt, :],
                        src_xt[:, mt, kh * 512:(kh + 1) * 512],
                        start=(mt == 0), stop=(mt == MT - 1))
                d = dst[:, kh * 512:(kh + 1) * 512]
                if b_name is not None and flags[b_name]:
                    nc.scalar.activation(d, pq, AF.Identity,
                                         bias=bias_tiles[b_name][:, h:h + 1])
                else:
                    nc.vector.tensor_copy(d, pq)

        def attention(src_q_xt, kv_xt, wq_name, bq_name, wk_name, bk_name,
                      vcat, ycat, post_head=None, v_cbs=None, pre_kh=None):
            """ycat[p(d), ht, k] (f32r) = per-head softmax(qk^T/sqrt(d)) v.

            exp runs on kt-PAIRS ([P,1024] ACT ops halve the fixed overhead),
            expq/vcat are bf16 (halves SBUF -> double-buffered across units),
            and the softmax denominator uses two column-packed 1-row matmul
            chains (out partitions 0/64 overlap on the PE) followed by tiny
            1-row reciprocals and a ones-row broadcast matmul -- this takes
            the old [128,512] DVE reciprocal (3.4us) off the critical path."""
            HQ = QW // 2
            v_cbs = list(v_cbs or [])
            pre_kh = pre_kh or {}
            with tc.tile_pool(name="attp", bufs=1) as attp, \
                    tc.tile_pool(name="wtq", bufs=6) as wtq:
                for h in range(H):
                    # this attention's own v-projection chunks are the
                    # earliest-ready PE work (per-kt input slices + small
                    # weight groups): run half before the first q-proj,
                    # the rest right after the first k-proj
                    if h == 0:
                        for _ in range(len(v_cbs) - len(v_cbs) // 2):
                            v_cbs.pop(0)()
                    qh = attp.tile([P, K], BF16, name=f"qh_{h}", tag="qh",
                                   bufs=2)
                    qk_head_proj(wtq, src_q_xt, wq_name, bq_name, h, qh)
                    if h in pre_kh:
                        kh = pre_kh[h]
                    else:
                        kh = attp.tile([P, K], BF16, name=f"kh_{h}",
                                       tag="kh", bufs=2)
                        qk_head_proj(wtq, kv_xt, wk_name, bk_name, h, kh)
                    if h == 0:
                        while v_cbs:
                            v_cbs.pop(0)()
                    for q in range(NQ):
                        expq = attp.tile([P, KT, QW], BF16,
                                         name=f"ex_{h}_{q}", tag="expq",
                                         bufs=2)
                        for kp in range(KT // 2):
                            sc = ps.tile([P, 2 * QW], F32,
                                         name=f"sc{h}_{q}_{kp}", tag="sc",
                                         bufs=2)
                            for j in range(2):
                                kt = 2 * kp + j
                                nc.tensor.matmul(
                                    sc[:, j * QW:(j + 1) * QW],
                                    kh[:, kt * P:(kt + 1) * P],
                                    qh[:, q * QW:(q + 1) * QW],
                                    start=True, stop=True)
                            nc.scalar.activation(
                                expq[:, 2 * kp:2 * kp + 2, :], sc, AF.Exp,
                                scale=ISQ)
                        # attnV first: its MMs gate only on expq (ACT), so
                        # the PE never waits behind the DVE denominator adds
                        psy = ps.tile([P, QW], F32, name=f"psy{h}_{q}",
                                      tag="ps")
                        for kt in range(KT):
                            nc.tensor.matmul(
                                psy, vcat[:, kt, h * DH:(h + 1) * DH],
                                expq[:, kt, :], start=(kt == 0),
                                stop=(kt == KT - 1))
                        # softmax denominator: two DVE pair-sums fold 8 kt
                        # tiles to 4, then a 4-long ones-matmul chain both
                        # reduces over partitions and broadcasts (PE cost
                        # halved vs the old 8-chain)
                        r0 = attp.tile([P, 2, QW], BF16, name=f"r0_{h}_{q}",
                                       tag="dred", bufs=2)
                        nc.vector.tensor_add(r0, expq[:, 0:2, :],
                                             expq[:, 2:4, :])
                        r1 = attp.tile([P, 2, QW], BF16, name=f"r1_{h}_{q}",
                                       tag="dred", bufs=2)
                        nc.vector.tensor_add(r1, expq[:, 4:6, :],
                                             expq[:, 6:8, :])
                        psd = ps.tile([P, QW], F32, name=f"psd{h}_{q}",
                                      tag="ps")
                        for j, (r, sl) in enumerate(
                                [(r0, 0), (r0, 1), (r1, 0), (r1, 1)]):
                            nc.tensor.matmul(psd, ones_b, r[:, sl, :],
                                             start=(j == 0), stop=(j == 3))
                        recip = attp.tile([P, QW], F32, name=f"rc_{h}_{q}",
                                          tag="recip", bufs=2)
                        nc.vector.reciprocal_approx_fast(recip, psd)
                        nc.vector.tensor_mul(
                            ycat[:, h, q * QW:(q + 1) * QW], psy, recip)
                    if post_head and h in post_head:
                        post_head[h]()

        def new_stats(kt):
            return statp.tile([P, 2, 6], F32, name=f"st{kt}", tag="stats")

        def ln_tail(z, kt, g_name, b_name, store_dram, to_out, stats=None):
            """x = LN(z) (+g/b); DMA to scratch or output; returns x tile.

            If `stats` is given, the per-half bn_stats were already emitted
            (overlapping the producer of the other half)."""
            if stats is None:
                stats = new_stats(kt)
                for sg in range(2):
                    nc.vector.bn_stats(out=stats[:, sg, :],
                                       in_=z[:, sg * 512:(sg + 1) * 512])
            mv = statp.tile([P, 2], F32, name=f"mv{kt}", tag="mv")
            nc.vector.bn_aggr(out=mv, in_=stats)
            std = statp.tile([P, 1], F32, name=f"sd{kt}", tag="std")
            nc.scalar.activation(std, mv[:, 1:2], AF.Sqrt, bias=eps_t)
            inv = statp.tile([P, 1], F32, name=f"iv{kt}", tag="inv")
            nc.vector.reciprocal(inv, std)
            x = natp.tile([P, M], F32, name=f"x{kt}", tag="x", bufs=2)
            nc.vector.tensor_scalar(out=x, in0=z, scalar1=mv[:, 0:1],
                                    scalar2=inv, op0=OP.subtract, op1=OP.mult)
            if flags[g_name]:
                nc.vector.tensor_mul(x, x, bias_tiles[g_name])
            if flags[b_name]:
                nc.vector.tensor_add(x, x, bias_tiles[b_name])
            if to_out:
                _ldq(nc).dma_start(out=out[kt * P:(kt + 1) * P, :], in_=x)
            else:
                _ldq(nc).dma_start(out=store_dram[kt * P:(kt + 1) * P, :],
                                   in_=x)
            return x

        def transpose_into(x, kt, xt_new):
            # quads: 4 transposes share one psum tile and one ACT cast --
            # 4x fewer psum-slot rotations and ACT ops, and a much shorter
            # serial chain after the last LayerNorm of a block
            for g in range(2):
                pt = ps.tile([P, 4 * P], F32, name=f"ptr{kt}_{g}", tag="ps")
                for j in range(4):
                    mt = 4 * g + j
                    nc.tensor.transpose(pt[:, j * P:(j + 1) * P],
                                        x[:, mt * P:(mt + 1) * P], ident)
                nc.scalar.activation(
                    xt_new[:, 4 * g:4 * g + 4, kt * P:(kt + 1) * P],
                    pt.rearrange("p (j d) -> p j d", j=4), AF.Copy)

        def wo_ln_block(ycat, wo_name, bo_name, resid_dram, g_name, b_name,
                        store_dram, xt_new, filler=(), filler_from=0,
                        filler_rate=5):
            """a = ycat @ Wo^T (+bo); z = resid + a; LN tail per kt.

            `filler` entries are (min_kt, cb) pairs of independent PE work
            (e.g. the next stage's v projection), interleaved a few per kt
            iteration once kt >= min_kt so the PE never starves on the LN
            chain."""
            filler = [f if isinstance(f, tuple) else (filler_from, f)
                      for f in filler]

            def drain(kt, n):
                for _ in range(n):
                    if filler and filler[0][0] <= kt:
                        filler.pop(0)[1]()
                    else:
                        break
            with tc.tile_pool(name="wop", bufs=8) as wop:
                wots = []
                for ht in range(HT):
                    w = wop.tile([P, M], BF16, name=f"{wo_name}{ht}",
                                 tag="wo8")
                    nc.sync.dma_start(out=w, in_=dram[wo_name][ht])
                    wots.append(w)
                x_prev = None
                for kt in range(KT):
                    if kt == 4 and filler_from == 0:
                        # Wo(kt4) waits on the attention q1 tail; fillers
                        # emitted after it cannot run during that wait, so
                        # drain a few ahead of it
                        drain(kt, filler_rate + 2)
                    resid = residp.tile([P, M], F32, name=f"rs{kt}",
                                        tag="resid")
                    _ldq(nc).dma_start(
                        out=resid, in_=resid_dram[kt * P:(kt + 1) * P, :])
                    z = natp.tile([P, M], F32, name=f"z{kt}", tag="z")
                    stats = new_stats(kt)
                    for mh in range(2):
                        pa = ps.tile([P, 512], F32, name=f"pa{kt}_{mh}",
                                     tag="ps")
                        for ht in range(HT):
                            nc.tensor.matmul(
                                pa, ycat[:, ht, kt * P:(kt + 1) * P],
                                wots[ht][:, mh * 512:(mh + 1) * 512],
                                start=(ht == 0), stop=(ht == HT - 1))
                        sl = slice(mh * 512, (mh + 1) * 512)
                        if flags[bo_name]:
                            nc.vector.scalar_tensor_tensor(
                                out=z[:, sl], in0=pa, scalar=1.0,
                                in1=bias_tiles[bo_name][:, sl],
                                op0=OP.bypass, op1=OP.add)
                            nc.vector.tensor_add(z[:, sl], z[:, sl],
                                                 resid[:, sl])
                        else:
                            nc.vector.tensor_add(z[:, sl], pa, resid[:, sl])
                        nc.vector.bn_stats(out=stats[:, mh, :], in_=z[:, sl])
                    x = ln_tail(z, kt, g_name, b_name, store_dram, False,
                                stats=stats)
                    drain(kt, filler_rate)
                    # transposes lag one kt so the PE never waits on the
                    # DVE LayerNorm chain of the current kt
                    if x_prev is not None:
                        transpose_into(x_prev, kt - 1, xt_new)
                    x_prev = x
                # run leftover fillers BEFORE the final transpose: the PE
                # queue is in-order, so independent work must precede the
                # transpose that waits on the last LN chain
                for _, cb in filler:
                    cb()
                transpose_into(x_prev, KT - 1, xt_new)

        # ================= self-attention =================
        # ycat_ca's pool opens before enc/vpool/khpre so those can close
        # right after ca_att (LIFO) while ycat survives into ca_wo_ln
        cay_ctx = tc.tile_pool(name="cay", bufs=1)
        cay = cay_ctx.__enter__()
        enc_ctx = tc.tile_pool(name="encp", bufs=1)
        encp = enc_ctx.__enter__()
        enc_xt = encp.tile([P, MT, K], BF16, name="encT", tag="enct")
        enc_src = dram["xt_enc"].rearrange("p (mt k) -> p mt k", mt=MT)
        vp_ctx = tc.tile_pool(name="vpool", bufs=1)
        vpool = vp_ctx.__enter__()

        khpre_ctx = tc.tile_pool(name="khpre", bufs=1)
        khpre = khpre_ctx.__enter__()
        kh_pre = {}

        with tc.tile_pool(name="sa_big", bufs=1) as bigp, \
                tc.tile_pool(name="wvsa", bufs=1) as wvp_sa:
            ycat = bigp.tile([P, HT, K], BF16, name="ycat_sa", tag="ycat")
            vcat_sa = vpool.tile([P, KT, HD], BF16, name="vcat_sa",
                                 tag="vcat")
            sa_v_cbs = v_proj_chunks(wvp_sa, xt, "wv_sa", "bv_sa", vcat_sa,
                                     W=256)

            def emit_enc_loads():
                # deferred enc loads: keeps the first ~20us of DMA bandwidth
                # for the sa_v/qk inputs; enc is first needed by the
                # v_proj_ca filler inside sa_wo_ln
                for kt in range(KT):
                    _ldq(nc).dma_start(
                        out=enc_xt[:, :, kt * P:(kt + 1) * P],
                        in_=enc_src[:, :, kt * P:(kt + 1) * P])

            with nc.named_scope("sa_att"):
                attention(xt, xt, "wq_sa", "bq_sa", "wk_sa", "bk_sa",
                          vcat_sa, ycat, post_head={2: emit_enc_loads},
                          v_cbs=sa_v_cbs)
            x1t = new_xt("x1T")
            vcat_ca = vpool.tile([P, KT, HD], BF16, name="vcat_ca",
                                 tag="vcat")
            with nc.named_scope("sa_wo_ln"), \
                    tc.tile_pool(name="wvca", bufs=1) as wvp, \
                    tc.tile_pool(name="wtqpre", bufs=2) as wtq_pre:
                filler = v_proj_chunks(wvp, enc_xt, "wv_ca", "bv_ca",
                                       vcat_ca)

                def make_kh_cb(h):
                    # prefetch the ca k-projection for head h: independent
                    # PE work that bridges the gap while the last LN /
                    # transpose of x1 drains on DVE
                    def cb():
                        t = khpre.tile([P, K], BF16, name=f"khpre_{h}",
                                       tag="khpre", bufs=4)
                        qk_head_proj(wtq_pre, enc_xt, "wk_ca", "bk_ca",
                                     h, t)
                        kh_pre[h] = t
                    return cb

                filler += [make_kh_cb(h) for h in range(4)]
                wo_ln_block(ycat, "wo_sa", "bo_sa", dram["dec_nat"],
                            "g1", "b1", x1_store, x1t, filler=filler,
                            filler_rate=4)
        xt = x1t

        # ================= cross-attention =================
        ycat = cay.tile([P, HT, K], BF16, name="ycat_ca", tag="ycat")
        with nc.named_scope("ca_att"):
            attention(xt, enc_xt, "wq_ca", "bq_ca", "wk_ca", "bk_ca",
                      vcat_ca, ycat, pre_kh=kh_pre)
        khpre_ctx.__exit__(None, None, None)
        vp_ctx.__exit__(None, None, None)
        enc_ctx.__exit__(None, None, None)

        # ffn pools open before ca_wo_ln: the k-half-0 W1 chunks run as PE
        # fillers inside it (their x2t k 0:512 inputs exist after the kt<=3
        # transposes), bridging the LN chain there AND removing the
        # ffn-start stall on the final transpose
        ffp_ctx = tc.tile_pool(name="ffp", bufs=1)
        ffp = ffp_ctx.__enter__()
        w1p_ctx = tc.tile_pool(name="w1p", bufs=10)
        w1p = w1p_ctx.__enter__()
        # rts[1] is allocated lazily at ffn start -- only rts[0] (written by
        # the ca_wo_ln W1 fillers) needs SBUF during the ca phase
        rts = [ffp.tile([P, FT, 512], BF16, name="rt0", tag="rt0"), None]
        x2t = new_xt("x2T")

        def w1_chunk_cb(ft, kqh):
            def cb():
                w1t = w1p.tile([P, MT, P], BF16,
                               name=f"w1_{ft}_{kqh}", tag="wt")
                nc.sync.dma_start(
                    out=w1t, in_=dram["w1"][ft].rearrange(
                        "p (mt d) -> p mt d", mt=MT))
                pf = ps.tile([P, 512], F32, name=f"pf{kqh}_{ft}",
                             tag="ps")
                for mt in range(MT):
                    nc.tensor.matmul(
                        pf, w1t[:, mt, :],
                        x2t[:, mt, kqh * 512:(kqh + 1) * 512],
                        start=(mt == 0), stop=(mt == MT - 1))
                if flags["bf1"]:
                    nc.scalar.activation(
                        rts[kqh][:, ft, :], pf, AF.Relu,
                        bias=bias_tiles["bf1"][:, ft:ft + 1])
                else:
                    nc.scalar.activation(rts[kqh][:, ft, :], pf,
                                         AF.Relu)
            return cb

        with nc.named_scope("ca_wo_ln"):
            wo_ln_block(ycat, "wo_ca", "bo_ca", x1_store, "g2", "b2",
                        x2_store, x2t,
                        filler=[w1_chunk_cb(ft, 0) for ft in range(FT)],
                        filler_from=5, filler_rate=8)
        xt = x2t

        # ================= feed-forward =================
        # k-half 0 of W1 already ran as ca_wo_ln fillers (rts[0] full).
        # Here: W1 k-half 1 (w1 re-streamed, bf16 so it is cheap) with the
        # mh=1 half of w2 preloading to SBUF behind it; then W2 per k-half:
        # a 4-deep streamed mh0 accumulation followed by per-ks mh1 chains
        # from SBUF whose LayerNorm tails overlap the next chain.
        with tc.tile_pool(name="ffw", bufs=8) as ffw, \
                tc.tile_pool(name="w2sbp", bufs=1) as w2sbp, \
                nc.named_scope("ffn"):
            rts[1] = w2sbp.tile([P, FT, 512], BF16, name="rt1",
                                tag="rt1")
            w2sb = w2sbp.tile([P, FT, 512], BF16, name="w2sb")
            # one w2sb preload DMA per W1 chunk: keeps the w1 ring fed (a
            # 4MB preload burst ahead of the w1t loads on the same sync
            # queue starves the ring)
            for ft in range(FT):
                w1_chunk_cb(ft, 1)()
                nc.sync.dma_start(out=w2sb[:, ft, :],
                                  in_=dram["w2"][ft, :, 512:])
            for kqh in range(2):
                rt = rts[kqh]
                x2r, st3 = {}, {}
                for ks in range(4):
                    kt = kqh * 4 + ks
                    r = w2sbp.tile([P, M], F32, name=f"x2r{kt}",
                                    tag="x2r", bufs=4)
                    _ldq(nc).dma_start(
                        out=r, in_=x2_store[kt * P:(kt + 1) * P, :])
                    x2r[ks] = r
                    st3[ks] = new_stats(kt)
                # mh0: stream w2 column-half 0, 4 k-tiles deep
                pacc = {ks: ps.tile([P, 512], F32, name=f"po{kqh}_{ks}",
                                    tag="ps") for ks in range(4)}
                for ft in range(FT):
                    w2t = ffw.tile([P, 512], BF16, name=f"w2_{kqh}_{ft}",
                                   tag="w2t")
                    nc.sync.dma_start(out=w2t,
                                      in_=dram["w2"][ft, :, 0:512])
                    for ks in range(4):
                        nc.tensor.matmul(
                            pacc[ks], rt[:, ft, ks * P:(ks + 1) * P], w2t,
                            start=(ft == 0), stop=(ft == FT - 1))
                for ks in range(4):
                    nc.vector.tensor_add(x2r[ks][:, 0:512], pacc[ks],
                                         x2r[ks][:, 0:512])
                    if flags["bf2"]:
                        nc.vector.tensor_add(x2r[ks][:, 0:512],
                                             x2r[ks][:, 0:512],
                                             bias_tiles["bf2"][:, 0:512])
                    nc.vector.bn_stats(out=st3[ks][:, 0, :],
                                       in_=x2r[ks][:, 0:512])
                # mh1: per-ks chains from preloaded SBUF weights; each ks's
                # LayerNorm + store drains while the next chain runs.  The
                # very last chain is split into two 256-wide halves so its
                # serial LN tail (the end of the kernel) is shorter.
                for ks in range(4):
                    last = (kqh == 1 and ks == 3)
                    if last:
                        st = statp.tile([P, 3, 6], F32, name="st_last",
                                        tag="stats3", bufs=1)
                        nc.vector.tensor_copy(st[:, 0, :], st3[ks][:, 0, :])
                        st3[ks] = st
                    for j, w in (((0, 512),) if not last
                                 else ((0, 256), (1, 256))):
                        lo = 512 + j * 256
                        pa1 = ps.tile([P, w], F32,
                                      name=f"pa1_{kqh}_{ks}_{j}", tag="ps")
                        for ft in range(FT):
                            nc.tensor.matmul(
                                pa1, rt[:, ft, ks * P:(ks + 1) * P],
                                w2sb[:, ft, j * 256:j * 256 + w],
                                start=(ft == 0), stop=(ft == FT - 1))
                        nc.vector.tensor_add(x2r[ks][:, lo:lo + w], pa1,
                                             x2r[ks][:, lo:lo + w])
                        if flags["bf2"]:
                            nc.vector.tensor_add(
                                x2r[ks][:, lo:lo + w],
                                x2r[ks][:, lo:lo + w],
                                bias_tiles["bf2"][:, lo:lo + w])
                        nc.vector.bn_stats(out=st3[ks][:, 1 + j, :],
                                           in_=x2r[ks][:, lo:lo + w])
                    ln_tail(x2r[ks], kqh * 4 + ks, "g3", "b3", None, True,
                            stats=st3[ks])
        w1p_ctx.__exit__(None, None, None)
        ffp_ctx.__exit__(None, None, None)
        cay_ctx.__exit__(None, None, None)


def _pack_inputs(inputs: dict):
    """Host-side packing -> (flags, per-core in_maps)."""
    f32 = np.float32
    dec = np.asarray(inputs["dec"], f32)
    enc = np.asarray(inputs["enc"], f32)

    def nz(x):
        return bool(np.any(np.asarray(x) != 0.0))

    flags = {
        "bq_sa": nz(inputs["bq_sa"]), "bk_sa": nz(inputs["bk_sa"]),
        "bv_sa": nz(inputs["bv_sa"]), "bo_sa": nz(inputs["bo_sa"]),
        "bq_ca": nz(inputs["bq_ca"]), "bk_ca": nz(inputs["bk_ca"]),
        "bv_ca": nz(inputs["bv_ca"]), "bo_ca": nz(inputs["bo_ca"]),
        "bf1": nz(inputs["bf1"]), "bf2": nz(inputs["bf2"]),
        "g1": bool(np.any(np.asarray(inputs["g1"]) != 1.0)),
        "b1": nz(inputs["b1"]),
        "g2": bool(np.any(np.asarray(inputs["g2"]) != 1.0)),
        "b2": nz(inputs["b2"]),
        "g3": bool(np.any(np.asarray(inputs["g3"]) != 1.0)),
        "b3": nz(inputs["b3"]),
    }

    def qk_pack(w):
        w = np.asarray(w, f32)  # (H, DH, M)
        return (w.transpose(0, 2, 1).reshape(H, MT, P, DH)
                .transpose(0, 2, 1, 3).reshape(H, P, MT * DH)
                .astype(ml_dtypes.bfloat16))

    def v_pack(w):
        w = np.asarray(w, f32)  # (H, DH, M) -> WvT [m, hd]
        wt_ = w.transpose(2, 0, 1).reshape(M, HD)
        return wt_.reshape(MT, P, HD).astype(ml_dtypes.bfloat16)

    def o_pack(w):  # (M, HD) -> WoT (HD, M) -> (HT, P, M)
        return (np.ascontiguousarray(np.asarray(w, f32).T).reshape(HT, P, M)
                .astype(ml_dtypes.bfloat16))

    W1 = np.asarray(inputs["W1"], f32)
    W2 = np.asarray(inputs["W2"], f32)
    shared = {
        "wq_sa": qk_pack(inputs["Wq_sa"]), "wk_sa": qk_pack(inputs["Wk_sa"]),
        "wv_sa": v_pack(inputs["Wv_sa"]), "wo_sa": o_pack(inputs["Wo_sa"]),
        "wq_ca": qk_pack(inputs["Wq_ca"]), "wk_ca": qk_pack(inputs["Wk_ca"]),
        "wv_ca": v_pack(inputs["Wv_ca"]), "wo_ca": o_pack(inputs["Wo_ca"]),
        "w1": (W1.reshape(FT, P, MT, P).transpose(0, 3, 2, 1)
               .reshape(FT, P, MT * P).astype(ml_dtypes.bfloat16)),
        "w2": np.ascontiguousarray(W2.T).reshape(FT, P, M)
        .astype(ml_dtypes.bfloat16),
    }
    for nm in ("bq_sa", "bk_sa", "bq_ca", "bk_ca"):
        if flags[nm]:
            shared[nm] = np.ascontiguousarray(np.asarray(inputs[nm], f32).T)
    for nm in ("bv_sa", "bv_ca"):
        if flags[nm]:
            shared[nm] = np.asarray(inputs[nm], f32).reshape(HD)
    for nm in ("bo_sa", "bo_ca", "bf2", "g1", "b1", "g2", "b2", "g3", "b3"):
        if flags[nm]:
            shared[nm] = np.asarray(inputs[nm], f32)
    if flags["bf1"]:
        shared["bf1"] = np.ascontiguousarray(
            np.asarray(inputs["bf1"], f32).reshape(FT, P).T)

    def xt_pack(x):  # (K, M) -> transposed, partition-contiguous (P, MT*K)
        return (x.T.reshape(MT, P, K).transpose(1, 0, 2)
                .reshape(P, MT * K).astype(ml_dtypes.bfloat16))

    in_maps = []
    for c in range(N_CORES):
        m = dict(shared)
        m["xt_dec"] = xt_pack(dec[c])
        m["xt_enc"] = xt_pack(enc[c])
        m["dec_nat"] = np.ascontiguousarray(dec[c])
        in_maps.append(m)
    return flags, in_maps


_NC_CACHE: dict = {}


def kernel(**inputs) -> np.ndarray:
    flags, in_maps = _pack_inputs(inputs)
    key = tuple(sorted(flags.items()))
    if key not in _NC_CACHE:
        _NC_CACHE[key] = build_kernel(flags)
    nc = _NC_CACHE[key]
    res = run_bass_kernel_spmd(nc, in_maps, core_ids=list(range(N_CORES)))
    return np.stack([res.results[c]["out"] for c in range(N_CORES)])

